# revision 24
# baseline (speedup 1.0000x reference)
"""Trainium-2 Bass kernel for nn_BoxRegressionLoss (greedy box matching + loss).

Contract: kernel(pred_boxes[8192,7] f32, gt_boxes[8192,7] f32) -> scalar f32
loss, numerically equal to the reference (sequential greedy nearest-center
matching with availability removal, then masked smooth-L1 / orientation /
BEV-IoU loss).

Single device launch (8 NeuronCores, pred rows sharded M/8 = 1024 per core):
the O(M*N) candidate search — all of the problem's FLOPs and memory traffic —
runs on device; the inherently sequential greedy walk (sanctioned host-side by
the spec hint) and the O(M) loss reduction run on the host from the device's
candidate lists.

Device program per core (preds in 64 spatially-tight blocks of 128 via a
lookahead median cut; each block scans the gts inside its bbox dilated by
D=2.0 m):
  1. TensorE: score(i,j) = -dist^2(i,j) as a K=16 bf16 matmul into PSUM
     (block-centered two-limb split => |score - exact| ~ 1e-3).
  2. Pool/DVE: two pairwise-max halving passes over the columns (PSUM->SBUF)
     so the expensive DVE MAX8/MAX_INDEX run on a quarter-width matrix; each
     surviving column represents 4 original columns (aliases).
  3. DVE: MAX8 + MAX_INDEX -> per-pred top-8 quarter-maxima + positions.

Host (exact, vectorized): expands each candidate into its 4 alias columns,
recomputes exact f32 reference distances for all of them, and runs the
reference-equivalent serial greedy: a pred matches its best available listed
candidate iff that beats the provable floor on every unlisted gt
(sqrt(-val[7]-EPS) for scanned columns, the dilation D for unscanned ones);
every ambiguous/conflicted/unmatched row degrades to an exact full-row
fallback, never to a wrong answer. Loss terms are the reference formulas in
f64 on the host; the final weighted sum is the gather/unshard step.
"""

import sys
import time as _time

sys.path.insert(0, "/opt/trn_rl_repo")

import numpy as np

import bass_rust as _br
import concourse.bass as bass
import concourse.mybir as mybir
from concourse import tile
from concourse.bass_utils import run_bass_kernel_spmd
from concourse.vector_clock import ScopedClock

# ----------------------------------------------------------------------------
# Compat patches for this container's walrus build, which rejects any
# instruction carrying more than one sync wait ("Too many sync wait commands").
# ----------------------------------------------------------------------------


def _drain_and_barrier_split(self, tick_clock, wait_clock):
    nc = self.nc
    drain_inst = nc.sync.drain()
    wait_clock.add_sem_waits(
        drain_inst.ins, ScopedClock({None: tick_clock.global_clock})
    )
    si = drain_inst.ins.sync_info
    waits = list(si.on_wait) if si is not None else []
    if len(waits) > 1:
        drain_inst.ins.sync_info = _br.SyncInfo(on_wait=[waits[0]], on_update=[])
        for w in waits[1:]:
            d2 = nc.sync.drain()
            d2.ins.sync_info = _br.SyncInfo(on_wait=[w], on_update=[])

    nc.all_engine_barrier(sem_only=False)
    popped = nc._tile_sem_poison_stack.pop()
    assert popped is self._sem_poison
    nc.clear_and_free_semaphores(list(self.sems.allocated().values()))
    nc.all_engine_barrier(sem_only=False)


tile.TileContext._drain_and_barrier = _drain_and_barrier_split

_WAITSPLIT_N = [0]


def _split_waits(nc, keep=1):
    for fn in nc.m.functions:
        for bb in fn.blocks:
            out = []
            changed = False
            for inst in bb.instructions:
                si = inst.sync_info
                waits = list(si.on_wait) if si is not None else []
                if len(waits) > keep:
                    changed = True
                    for w in waits[: len(waits) - keep]:
                        ev = mybir.InstEventSemaphore(
                            name=f"waitsplit-{_WAITSPLIT_N[0]}", ins=[], outs=[]
                        )
                        _WAITSPLIT_N[0] += 1
                        ev.engine = inst.engine
                        ev.sync_info = _br.SyncInfo(on_wait=[w], on_update=[])
                        out.append(ev)
                    inst.sync_info = _br.SyncInfo(
                        on_wait=waits[len(waits) - keep :],
                        on_update=list(si.on_update),
                    )
                out.append(inst)
            if changed:
                bb.instructions = out


# ----------------------------------------------------------------------------
# Problem constants (hardcoded per the task spec)
# ----------------------------------------------------------------------------
M = 8192
N = 8192
N_CORES = 8
M_PER_CORE = M // N_CORES            # 1024
N_SLOTS = M_PER_CORE // 128          # 8
N_BLOCKS = M // 128                  # 64
K_CAND = 8
N_ALIAS = 4                          # two halving levels -> 4 cols per winner
MATCH_THRESH = 5.0
DILATE = 1.25                        # scan radius; unscanned gts are >= D away
W_CENTER, W_SIZE, W_IOU = 1.0, 0.5, 2.0
K_ROWS = 16                          # 12 cross products + 2 |g|^2 + 2 |p|^2
SENT_OFF = 1.0e4                     # sentinel gt offset => score ~ -3e8
MAX_TIER = 512                       # matmul moving-dim / PSUM bank limit

F32 = mybir.dt.float32
BF16 = mybir.dt.bfloat16
U16 = mybir.dt.uint16
OP = mybir.AluOpType

LAST_EXEC_NS = {"phase1": None}
TRACE = False
DIAG = {}

_PROGRAMS = {}


# ----------------------------------------------------------------------------
# Device program: per-slot matmul scores -> 2 halving passes -> MAX8/MAX_INDEX
# ----------------------------------------------------------------------------
def _build_program(tiers):
    """Per slot: matmul scores -> ACT copy PSUM->SBUF bf16 -> DVE halving x2
    at the 2x 16-bit rate -> DVE MAX8 + MAX_INDEX on the quarter-width tile.

    GPSIMD has no general tensor ops on TRN2 and DVE reads at most one PSUM
    operand, so the score matrix is staged through one bf16 ACT copy; ACT and
    PE run ahead of the serial DVE chain."""
    tiers = tuple(int(t) for t in tiers)
    C = sum(tiers)
    off = np.concatenate([[0], np.cumsum(tiers)]).astype(int)

    nc = bass.Bass("TRN2", target_bir_lowering=False, debug=False)
    # split input: A = pred limbs + slot-0 gt columns (critical path),
    # B = remaining gt columns, DMA'd in parallel on another queue
    ca = M_PER_CORE + tiers[0]
    limbsA = nc.dram_tensor("limbsA", [K_ROWS, ca], BF16, kind="ExternalInput")
    limbsB = nc.dram_tensor(
        "limbsB", [K_ROWS, M_PER_CORE + C - ca], BF16, kind="ExternalInput"
    )
    # one packed output DMA: cols 0..63 = top-8 vals (bf16 bitcast),
    # cols 64..127 = their quarter-tile positions (u16)
    out = nc.dram_tensor(
        "out", [128, 2 * N_SLOTS * K_CAND], U16, kind="ExternalOutput"
    )

    with tile.TileContext(nc) as tc:
        with (
            tc.tile_pool(name="w", bufs=1) as wpool,
            tc.tile_pool(name="hq", bufs=3) as hqpool,
            tc.tile_pool(name="ps", bufs=4, space="PSUM") as ppool,
        ):
            lt = wpool.tile([K_ROWS, M_PER_CORE + C], BF16)
            nc.sync.dma_start(out=lt[:, 0:ca], in_=limbsA[:])
            nc.scalar.dma_start(out=lt[:, ca:], in_=limbsB[:])

            ot = wpool.tile([128, 2 * N_SLOTS * K_CAND], U16)
            NV = N_SLOTS * K_CAND

            # slots processed in pairs: one PSUM tile + one ACT copy per
            # pair; equal-tier pairs merge the halving passes into single
            # 3D-AP TensorTensor ops
            for p in range(N_SLOTS // 2):
                s0, s1 = 2 * p, 2 * p + 1
                B0, B1 = tiers[s0], tiers[s1]
                assert B0 % 4 == 0 and B1 % 4 == 0 and B0 + B1 <= MAX_TIER
                ps = ppool.tile([128, B0 + B1], F32, tag="ps")
                for s, c0, c1 in ((s0, 0, B0), (s1, B0, B0 + B1)):
                    nc.tensor.matmul(
                        ps[:, c0:c1],
                        lt[:, s * 128 : (s + 1) * 128],
                        lt[:, M_PER_CORE + off[s] : M_PER_CORE + off[s] + tiers[s]],
                        start=True,
                        stop=True,
                    )
                scp = hqpool.tile([128, B0 + B1], BF16, tag="scp")
                nc.scalar.copy(scp[:], ps[:])

                if B0 == B1:
                    B = B0
                    H, Q = B // 2, B // 4
                    s3 = scp[:].rearrange("a (s c) -> a s c", s=2)
                    ht = hqpool.tile([128, 2, H], BF16, tag="h")
                    nc.vector.tensor_tensor(
                        out=ht[:], in0=s3[:, :, 0:H], in1=s3[:, :, H:B], op=OP.max
                    )
                    qt = hqpool.tile([128, 2, Q], BF16, tag="q")
                    nc.vector.tensor_tensor(
                        out=qt[:], in0=ht[:, :, 0:Q], in1=ht[:, :, Q:H], op=OP.max
                    )
                    qts = (qt[:, 0, :], qt[:, 1, :])
                else:
                    qts = []
                    for s, c0 in ((s0, 0), (s1, B0)):
                        B = tiers[s]
                        H, Q = B // 2, B // 4
                        ht = hqpool.tile([128, H], BF16, tag=f"h{s % 2}")
                        nc.vector.tensor_tensor(
                            out=ht[:],
                            in0=scp[:, c0 : c0 + H],
                            in1=scp[:, c0 + H : c0 + B],
                            op=OP.max,
                        )
                        qt = hqpool.tile([128, Q], BF16, tag=f"q{s % 2}")
                        nc.vector.tensor_tensor(
                            out=qt[:], in0=ht[:, 0:Q], in1=ht[:, Q:H], op=OP.max
                        )
                        qts.append(qt[:])

                for s, qv in ((s0, qts[0]), (s1, qts[1])):
                    vv = ot[:, s * K_CAND : (s + 1) * K_CAND].bitcast(BF16)
                    iv = ot[:, NV + s * K_CAND : NV + (s + 1) * K_CAND]
                    nc.vector.max(out=vv, in_=qv)
                    nc.vector.max_index(out=iv, in_max=vv, in_values=qv)

            nc.sync.dma_start(out=out[:], in_=ot[:])
    return nc


def _get_program(tiers):
    key = tuple(int(t) for t in tiers)
    if key not in _PROGRAMS:
        nc = _build_program(key)
        _split_waits(nc)
        _PROGRAMS[key] = nc
    return _PROGRAMS[key]


# ----------------------------------------------------------------------------
# Host-side spatial partitioning: lookahead median cut (pick the split axis
# minimizing the children's scanned-gt total).
# ----------------------------------------------------------------------------
def _lookahead_cut(p3, g3):
    def gcount(idx):
        pts = p3[idx]
        lo = pts.min(axis=0) - DILATE
        hi = pts.max(axis=0) + DILATE
        return int((((g3 >= lo) & (g3 <= hi)).all(axis=1)).sum())

    def rec(idx, depth):
        if depth == 0:
            return [idx]
        q = p3[idx]
        k = len(idx) // 2
        best, bestc = None, 1 << 60
        for d in range(3):
            part = np.argpartition(q[:, d], k)
            a, b = idx[part[:k]], idx[part[k:]]
            c = gcount(a) + gcount(b)
            if c < bestc:
                bestc, best = c, (a, b)
        return rec(best[0], depth - 1) + rec(best[1], depth - 1)

    return rec(np.arange(M), int(np.log2(N_BLOCKS)))


def _split2_bf16(x):
    """Split f64 array into two bf16 limbs (~16 mantissa bits total)."""
    import ml_dtypes

    bf = ml_dtypes.bfloat16
    h = x.astype(bf)
    m = (x - h.astype(np.float64)).astype(bf)
    return h, m


# ----------------------------------------------------------------------------
# Host-side exact greedy walk (serial dictatorship == reference lax.scan)
# ----------------------------------------------------------------------------
def _host_greedy(pred, gt, srt_d, srt_g, floor2, bad):
    """srt_d [M,32] exact f32 candidate distances sorted asc (inf = sentinel),
    srt_g [M,32] matching global gt ids (-1 = sentinel), floor2 [M] lower
    bound on dist^2 of any gt NOT in the candidate list, bad [M] rows that
    must take the exact fallback."""
    p3 = pred[:, :3].astype(np.float32)
    g3 = gt[:, :3].astype(np.float32)

    avail = np.ones(N, dtype=bool)
    mask = np.zeros(M, dtype=bool)
    sel = np.zeros(M, dtype=np.int64)
    n_fallback = 0

    def exact_row_step(i):
        diff_i = p3[i][None, :] - g3
        d2_i = np.sum(diff_i * diff_i, axis=-1, dtype=np.float32)
        drow = np.sqrt(d2_i, dtype=np.float32)
        dm = np.where(avail, drow, np.inf)
        j = int(np.argmin(dm))
        return j, bool(dm[j] < MATCH_THRESH)

    d_l = srt_d.tolist()
    g_l = srt_g.tolist()
    f_l = floor2.tolist()
    b_l = bad.tolist()
    INF = float("inf")

    for i in range(M):
        j = -1
        ok = False
        need_fb = b_l[i]
        if not need_fb:
            row_d, row_g = d_l[i], g_l[i]
            dk, gk = INF, -1
            for k in range(len(row_g)):
                g = row_g[k]
                if g < 0:
                    break
                if avail[g]:
                    dk, gk = row_d[k], g
                    break
            if (
                gk >= 0
                and dk < MATCH_THRESH
                and dk < DILATE
                and dk * dk < f_l[i]
            ):
                j, ok = gk, True
            else:
                need_fb = True
        if need_fb:
            j, ok = exact_row_step(i)
            n_fallback += 1
        sel[i] = j
        mask[i] = ok
        if ok:
            avail[j] = False

    return mask, sel, n_fallback


# ----------------------------------------------------------------------------
# Host-side loss (reference formulas, f64)
# ----------------------------------------------------------------------------
def _host_loss(pred, gt, mask, sel):
    pb = pred.astype(np.float64)
    mg = gt[sel].astype(np.float64)
    m = mask.astype(np.float64)
    k = max(m.sum(), 1.0)

    def sl1(x):
        a = np.abs(x)
        return np.where(a < 1.0, 0.5 * a * a, a - 0.5)

    lc = (m[:, None] * sl1(pb[:, :3] - mg[:, :3])).sum() / (3 * k)
    ls = (m[:, None] * sl1(pb[:, 3:6] - mg[:, 3:6])).sum() / (3 * k)
    d = pb[:, 6] - mg[:, 6]
    d = np.arctan2(np.sin(d), np.cos(d))
    lo = (m * sl1(d)).sum() / k
    x1, y1, l1, w1 = pb[:, 0], pb[:, 1], pb[:, 3], pb[:, 4]
    x2, y2, l2, w2 = mg[:, 0], mg[:, 1], mg[:, 3], mg[:, 4]
    iw = np.clip(
        np.minimum(x1 + l1 / 2, x2 + l2 / 2) - np.maximum(x1 - l1 / 2, x2 - l2 / 2),
        0, None,
    )
    ih = np.clip(
        np.minimum(y1 + w1 / 2, y2 + w2 / 2) - np.maximum(y1 - w1 / 2, y2 - w2 / 2),
        0, None,
    )
    inter = iw * ih
    union = l1 * w1 + l2 * w2 - inter
    iou = inter / (union + 1e-6)
    li = (m * (1.0 - iou)).sum() / k
    return W_CENTER * lc + W_SIZE * (ls + lo) + W_IOU * li


# ----------------------------------------------------------------------------
# Main entry point
# ----------------------------------------------------------------------------
def kernel(pred_boxes: np.ndarray, gt_boxes: np.ndarray) -> np.ndarray:
    pred = np.ascontiguousarray(np.asarray(pred_boxes, dtype=np.float32))
    gt = np.ascontiguousarray(np.asarray(gt_boxes, dtype=np.float32))
    assert pred.shape == (M, 7) and gt.shape == (N, 7)
    core_ids = list(range(N_CORES))

    p3 = pred[:, :3].astype(np.float64)
    g3 = gt[:, :3].astype(np.float64)

    # ---- spatial blocks + per-block scanned-gt selection ----
    blocks = _lookahead_cut(p3, g3)
    insides, centers = [], []
    for blk in blocks:
        pts = p3[blk]
        lo = pts.min(axis=0) - DILATE
        hi = pts.max(axis=0) + DILATE
        insides.append(np.nonzero(((g3 >= lo) & (g3 <= hi)).all(axis=1))[0])
        centers.append(0.5 * (pts.min(axis=0) + pts.max(axis=0)))
    counts = np.array([len(x) for x in insides])
    ranked = np.argsort(-counts, kind="stable")
    # slot budgets ascending: the pipeline-fill cost is set by slot 0, so the
    # smallest blocks go first
    assign = ranked.reshape(N_SLOTS, N_CORES)[::-1]  # [slot, core] -> block id

    # per-slot budgets from the data (pad to 32; each slot PAIR shares one
    # PSUM bank, so pair sums are capped at MAX_TIER f32 columns)
    tiers = []
    for s in range(N_SLOTS):
        mx = int(counts[assign[s]].max())
        tiers.append(min(MAX_TIER - 32, max(32, -(-mx // 32) * 32)))
    for s0 in range(0, N_SLOTS, 2):
        if tiers[s0] + tiers[s0 + 1] > MAX_TIER:
            tiers[s0] = max(32, MAX_TIER - tiers[s0 + 1])
    overflow = np.zeros((N_CORES, N_SLOTS), dtype=bool)
    for s in range(N_SLOTS):
        for c in core_ids:
            if counts[assign[s, c]] > tiers[s]:
                overflow[c, s] = True
    tiers = tuple(tiers)
    off = np.concatenate([[0], np.cumsum(tiers)]).astype(int)
    C = int(off[-1])

    # ---- build per-core limb tensors ----
    idx_map = np.zeros((N_CORES, N_SLOTS, max(tiers)), dtype=np.int64)
    sent_mask = np.ones((N_CORES, N_SLOTS, max(tiers)), dtype=bool)
    in_maps = []
    import ml_dtypes

    bf = ml_dtypes.bfloat16
    for c in core_ids:
        arr = np.zeros((K_ROWS, M_PER_CORE + C), dtype=bf)
        for s in range(N_SLOTS):
            bi = assign[s, c]
            c0 = centers[bi]
            B = tiers[s]
            inside = insides[bi][:B]
            n = len(inside)
            idx_map[c, s, :n] = inside
            sent_mask[c, s, :n] = False

            # pred side: 128 preds of the block
            pc = 2.0 * (p3[blocks[bi]] - c0)          # [128, 3]
            ph, pm = _split2_bf16(pc)
            pn = -np.sum((0.5 * pc) ** 2, axis=1)     # -|p'|^2  [128]
            pnh, pnm = _split2_bf16(pn)
            colp = slice(s * 128, (s + 1) * 128)
            for cc in range(3):
                arr[cc * 4 + 0, colp] = ph[:, cc]
                arr[cc * 4 + 1, colp] = ph[:, cc]
                arr[cc * 4 + 2, colp] = pm[:, cc]
                arr[cc * 4 + 3, colp] = pm[:, cc]
            arr[12, colp] = 1.0
            arr[13, colp] = 1.0
            arr[14, colp] = pnh
            arr[15, colp] = pnm

            # gt side: scanned gts then sentinels
            gc = np.full((B, 3), SENT_OFF, dtype=np.float64)
            gc[:n] = g3[inside] - c0
            gh, gm = _split2_bf16(gc)
            gn = -np.sum(gc * gc, axis=1)             # -|g'|^2  [B]
            gnh, gnm = _split2_bf16(gn)
            colg = slice(M_PER_CORE + off[s], M_PER_CORE + off[s] + B)
            for cc in range(3):
                arr[cc * 4 + 0, colg] = gh[:, cc]
                arr[cc * 4 + 1, colg] = gm[:, cc]
                arr[cc * 4 + 2, colg] = gh[:, cc]
                arr[cc * 4 + 3, colg] = gm[:, cc]
            arr[12, colg] = gnh
            arr[13, colg] = gnm
            arr[14, colg] = 1.0
            arr[15, colg] = 1.0
        ca = M_PER_CORE + tiers[0]
        in_maps.append(
            {
                "limbsA": np.ascontiguousarray(arr[:, 0:ca]),
                "limbsB": np.ascontiguousarray(arr[:, ca:]),
            }
        )

    perm = np.concatenate(
        [blocks[assign[s, c]] for c in core_ids for s in range(N_SLOTS)]
    )

    # ---- device launch ----
    nc = _get_program(tiers)
    res = run_bass_kernel_spmd(nc, in_maps, core_ids, trace=TRACE)
    LAST_EXEC_NS["phase1"] = res.exec_time_ns

    import ml_dtypes as _mld

    NV = N_SLOTS * K_CAND
    vals_p = np.concatenate(
        [
            np.ascontiguousarray(res.results[c]["out"][:, :NV])
            .view(_mld.bfloat16)
            .reshape(128, N_SLOTS, K_CAND)
            .transpose(1, 0, 2)
            .reshape(M_PER_CORE, K_CAND)
            for c in core_ids
        ],
        axis=0,
    )
    idxs_p = np.concatenate(
        [
            res.results[c]["out"][:, NV:]
            .reshape(128, N_SLOTS, K_CAND)
            .transpose(1, 0, 2)
            .reshape(M_PER_CORE, K_CAND)
            for c in core_ids
        ],
        axis=0,
    )

    # ---- decode: expand each winner into its 4 alias columns ----
    core_of_row = np.repeat(np.arange(N_CORES), M_PER_CORE)
    slot_of_row = np.tile(np.repeat(np.arange(N_SLOTS), 128), N_CORES)
    tiers_arr = np.array(tiers)
    q_of_row = tiers_arr[slot_of_row] // 4                    # [M]
    loc_raw = idxs_p.astype(np.int64)                         # [M, 8] in [0,Q)
    loc = np.minimum(loc_raw, q_of_row[:, None] - 1)
    alias = loc[:, :, None] + np.arange(N_ALIAS)[None, None, :] * q_of_row[
        :, None, None
    ]                                                          # [M, 8, 4]
    gids = idx_map[core_of_row[:, None, None], slot_of_row[:, None, None], alias]
    sent = sent_mask[core_of_row[:, None, None], slot_of_row[:, None, None], alias]
    sent |= (loc_raw != loc)[:, :, None]

    p3f = pred[:, :3].astype(np.float32)
    g3f = gt[:, :3].astype(np.float32)
    diffc = p3f[perm][:, None, None, :] - g3f[gids]
    d2c = np.sum(diffc * diffc, axis=-1, dtype=np.float32)
    dc = np.sqrt(d2c, dtype=np.float32)
    dc[sent] = np.inf
    gids_s = np.where(sent, -1, gids)

    # empirical score-error bound: approx val vs exact best-alias score.
    # bf16 staging makes the error value-relative: err <= e_abs + REL*|val|.
    vap = vals_p.astype(np.float64)                            # approx max score
    d2min = np.min(np.where(sent, np.inf, d2c.astype(np.float64)), axis=2)
    real = np.isfinite(d2min) & (vap > -1.0e8)
    err = np.abs(np.where(real, -vap - d2min, 0.0))
    REL = 2.0 ** -8
    e_abs = float(np.maximum(err - REL * np.abs(vap), 0.0).max())
    DIAG["eps_abs"] = e_abs

    # flatten to [M, 32] sorted by (exact distance, gt id)
    dflat = dc.reshape(M, K_CAND * N_ALIAS)
    gflat = gids_s.reshape(M, K_CAND * N_ALIAS)
    order = np.lexsort((gflat, dflat), axis=-1)
    srt_d = np.take_along_axis(dflat, order, axis=-1)
    srt_g = np.take_along_axis(gflat, order, axis=-1)
    srt_g[~np.isfinite(srt_d)] = -1
    # move sentinels (-1 gid) to the end marker-wise: walk breaks at g<0, so
    # ensure no real candidate sorts after a sentinel (inf distance => last).
    v7 = vap[:, K_CAND - 1]
    eps_row = 1.3 * (e_abs + REL * np.abs(v7)) + 1e-3
    floor2 = np.maximum(-v7 - eps_row, 0.0)
    bad = overflow[core_of_row, slot_of_row]

    # back to original pred order
    inv = np.empty(M, dtype=np.int64)
    inv[perm] = np.arange(M)
    srt_d = srt_d[inv]
    srt_g = srt_g[inv]
    floor2 = floor2[inv]
    bad = bad[inv]

    t_walk = _time.time()
    mask, sel, n_fb = _host_greedy(pred, gt, srt_d, srt_g, floor2, bad)
    DIAG["n_fallback"] = n_fb
    DIAG["n_overflow_blocks"] = int(overflow.sum())
    DIAG["t_walk"] = _time.time() - t_walk
    DIAG["tiers"] = tiers

    loss = _host_loss(pred, gt, mask, sel)
    return np.float32(loss)


# revision 28
# speedup vs baseline: 1.0365x; 1.0365x over previous
"""Trainium-2 Bass kernel for nn_BoxRegressionLoss (greedy box matching + loss).

Contract: kernel(pred_boxes[8192,7] f32, gt_boxes[8192,7] f32) -> scalar f32
loss, numerically equal to the reference (sequential greedy nearest-center
matching with availability removal, then masked smooth-L1 / orientation /
BEV-IoU loss).

Single device launch (8 NeuronCores, pred rows sharded M/8 = 1024 per core):
the O(M*N) candidate search — all of the problem's FLOPs and memory traffic —
runs on device; the inherently sequential greedy walk (sanctioned host-side by
the spec hint) and the O(M) loss reduction run on the host from the device's
candidate lists.

Device program per core (preds in 64 spatially-tight blocks of 128 via a
lookahead median cut; each block scans the gts inside its bbox dilated by
D=2.0 m):
  1. TensorE: score(i,j) = -dist^2(i,j) as a K=16 bf16 matmul into PSUM
     (block-centered two-limb split => |score - exact| ~ 1e-3).
  2. Pool/DVE: two pairwise-max halving passes over the columns (PSUM->SBUF)
     so the expensive DVE MAX8/MAX_INDEX run on a quarter-width matrix; each
     surviving column represents 4 original columns (aliases).
  3. DVE: MAX8 + MAX_INDEX -> per-pred top-8 quarter-maxima + positions.

Host (exact, vectorized): expands each candidate into its 4 alias columns,
recomputes exact f32 reference distances for all of them, and runs the
reference-equivalent serial greedy: a pred matches its best available listed
candidate iff that beats the provable floor on every unlisted gt
(sqrt(-val[7]-EPS) for scanned columns, the dilation D for unscanned ones);
every ambiguous/conflicted/unmatched row degrades to an exact full-row
fallback, never to a wrong answer. Loss terms are the reference formulas in
f64 on the host; the final weighted sum is the gather/unshard step.
"""

import sys
import time as _time

sys.path.insert(0, "/opt/trn_rl_repo")

import numpy as np

import bass_rust as _br
import concourse.bass as bass
import concourse.mybir as mybir
from concourse import tile
from concourse.bass_utils import run_bass_kernel_spmd
from concourse.vector_clock import ScopedClock

# ----------------------------------------------------------------------------
# Compat patches for this container's walrus build, which rejects any
# instruction carrying more than one sync wait ("Too many sync wait commands").
# ----------------------------------------------------------------------------


def _drain_and_barrier_split(self, tick_clock, wait_clock):
    nc = self.nc
    drain_inst = nc.sync.drain()
    wait_clock.add_sem_waits(
        drain_inst.ins, ScopedClock({None: tick_clock.global_clock})
    )
    si = drain_inst.ins.sync_info
    waits = list(si.on_wait) if si is not None else []
    if len(waits) > 1:
        drain_inst.ins.sync_info = _br.SyncInfo(on_wait=[waits[0]], on_update=[])
        for w in waits[1:]:
            d2 = nc.sync.drain()
            d2.ins.sync_info = _br.SyncInfo(on_wait=[w], on_update=[])

    nc.all_engine_barrier(sem_only=False)
    popped = nc._tile_sem_poison_stack.pop()
    assert popped is self._sem_poison
    nc.clear_and_free_semaphores(list(self.sems.allocated().values()))
    nc.all_engine_barrier(sem_only=False)


tile.TileContext._drain_and_barrier = _drain_and_barrier_split

_WAITSPLIT_N = [0]


def _split_waits(nc, keep=1):
    for fn in nc.m.functions:
        for bb in fn.blocks:
            out = []
            changed = False
            for inst in bb.instructions:
                si = inst.sync_info
                waits = list(si.on_wait) if si is not None else []
                if len(waits) > keep:
                    changed = True
                    for w in waits[: len(waits) - keep]:
                        ev = mybir.InstEventSemaphore(
                            name=f"waitsplit-{_WAITSPLIT_N[0]}", ins=[], outs=[]
                        )
                        _WAITSPLIT_N[0] += 1
                        ev.engine = inst.engine
                        ev.sync_info = _br.SyncInfo(on_wait=[w], on_update=[])
                        out.append(ev)
                    inst.sync_info = _br.SyncInfo(
                        on_wait=waits[len(waits) - keep :],
                        on_update=list(si.on_update),
                    )
                out.append(inst)
            if changed:
                bb.instructions = out


# ----------------------------------------------------------------------------
# Problem constants (hardcoded per the task spec)
# ----------------------------------------------------------------------------
M = 8192
N = 8192
N_CORES = 8
M_PER_CORE = M // N_CORES            # 1024
N_SLOTS = M_PER_CORE // 128          # 8
N_BLOCKS = M // 128                  # 64
K_CAND = 8
N_ALIAS = 4                          # two halving levels -> 4 cols per winner
MATCH_THRESH = 5.0
DILATE = 1.25                        # scan radius; unscanned gts are >= D away
W_CENTER, W_SIZE, W_IOU = 1.0, 0.5, 2.0
K_ROWS = 16                          # 12 cross products + 2 |g|^2 + 2 |p|^2
SENT_OFF = 1.0e4                     # sentinel gt offset => score ~ -3e8
MAX_TIER = 512                       # matmul moving-dim / PSUM bank limit

F32 = mybir.dt.float32
BF16 = mybir.dt.bfloat16
U16 = mybir.dt.uint16
OP = mybir.AluOpType

LAST_EXEC_NS = {"phase1": None}
TRACE = False
DIAG = {}

_PROGRAMS = {}


# ----------------------------------------------------------------------------
# Device program: per-slot matmul scores -> 2 halving passes -> MAX8/MAX_INDEX
# ----------------------------------------------------------------------------
def _build_program(tiers):
    """Per slot: matmul scores -> ACT copy PSUM->SBUF bf16 -> DVE halving x2
    at the 2x 16-bit rate -> DVE MAX8 + MAX_INDEX on the quarter-width tile.

    GPSIMD has no general tensor ops on TRN2 and DVE reads at most one PSUM
    operand, so the score matrix is staged through one bf16 ACT copy; ACT and
    PE run ahead of the serial DVE chain."""
    tiers = tuple(int(t) for t in tiers)
    C = sum(tiers)
    off = np.concatenate([[0], np.cumsum(tiers)]).astype(int)

    nc = bass.Bass("TRN2", target_bir_lowering=False, debug=False)
    limbs = nc.dram_tensor(
        "limbs", [K_ROWS, M_PER_CORE + C], BF16, kind="ExternalInput"
    )
    # one packed output DMA: cols 0..63 = top-8 vals (bf16 bitcast),
    # cols 64..127 = their quarter-tile positions (u16)
    out = nc.dram_tensor(
        "out", [128, 2 * N_SLOTS * K_CAND], U16, kind="ExternalOutput"
    )

    with tile.TileContext(nc) as tc:
        with (
            tc.tile_pool(name="w", bufs=1) as wpool,
            tc.tile_pool(name="hq", bufs=3) as hqpool,
            tc.tile_pool(name="ps", bufs=4, space="PSUM") as ppool,
        ):
            lt = wpool.tile([K_ROWS, M_PER_CORE + C], BF16)
            nc.sync.dma_start(out=lt[:], in_=limbs[:])

            ot = wpool.tile([128, 2 * N_SLOTS * K_CAND], U16)
            NV = N_SLOTS * K_CAND

            for s in range(N_SLOTS):
                B = tiers[s]
                assert B % 4 == 0 and B <= MAX_TIER
                H, Q = B // 2, B // 4
                ps = ppool.tile([128, B], F32, tag="ps")
                nc.tensor.matmul(
                    ps[:],
                    lt[:, s * 128 : (s + 1) * 128],
                    lt[:, M_PER_CORE + off[s] : M_PER_CORE + off[s] + B],
                    start=True,
                    stop=True,
                )
                scp = hqpool.tile([128, B], BF16, tag="scp")
                nc.scalar.copy(scp[:], ps[:])
                ht = hqpool.tile([128, H], BF16, tag="h")
                nc.vector.tensor_tensor(
                    out=ht[:], in0=scp[:, 0:H], in1=scp[:, H:B], op=OP.max
                )
                qt = hqpool.tile([128, Q], BF16, tag="q")
                nc.vector.tensor_tensor(
                    out=qt[:], in0=ht[:, 0:Q], in1=ht[:, Q:H], op=OP.max
                )
                vv = ot[:, s * K_CAND : (s + 1) * K_CAND].bitcast(BF16)
                iv = ot[:, NV + s * K_CAND : NV + (s + 1) * K_CAND]
                nc.vector.max(out=vv, in_=qt[:])
                nc.vector.max_index(out=iv, in_max=vv, in_values=qt[:])

            nc.sync.dma_start(out=out[:], in_=ot[:])
    return nc


def _get_program(tiers):
    key = tuple(int(t) for t in tiers)
    if key not in _PROGRAMS:
        nc = _build_program(key)
        _split_waits(nc)
        _PROGRAMS[key] = nc
    return _PROGRAMS[key]


# ----------------------------------------------------------------------------
# Host-side spatial partitioning: lookahead median cut (pick the split axis
# minimizing the children's scanned-gt total).
# ----------------------------------------------------------------------------
def _lookahead_cut(p3, g3):
    def gcount(idx):
        pts = p3[idx]
        lo = pts.min(axis=0) - DILATE
        hi = pts.max(axis=0) + DILATE
        return int((((g3 >= lo) & (g3 <= hi)).all(axis=1)).sum())

    def rec(idx, depth):
        if depth == 0:
            return [idx]
        q = p3[idx]
        k = len(idx) // 2
        best, bestc = None, 1 << 60
        for d in range(3):
            part = np.argpartition(q[:, d], k)
            a, b = idx[part[:k]], idx[part[k:]]
            c = gcount(a) + gcount(b)
            if c < bestc:
                bestc, best = c, (a, b)
        return rec(best[0], depth - 1) + rec(best[1], depth - 1)

    return rec(np.arange(M), int(np.log2(N_BLOCKS)))


def _split2_bf16(x):
    """Split f64 array into two bf16 limbs (~16 mantissa bits total)."""
    import ml_dtypes

    bf = ml_dtypes.bfloat16
    h = x.astype(bf)
    m = (x - h.astype(np.float64)).astype(bf)
    return h, m


# ----------------------------------------------------------------------------
# Host-side exact greedy walk (serial dictatorship == reference lax.scan)
# ----------------------------------------------------------------------------
def _host_greedy(pred, gt, srt_d, srt_g, floor2, bad):
    """srt_d [M,32] exact f32 candidate distances sorted asc (inf = sentinel),
    srt_g [M,32] matching global gt ids (-1 = sentinel), floor2 [M] lower
    bound on dist^2 of any gt NOT in the candidate list, bad [M] rows that
    must take the exact fallback."""
    p3 = pred[:, :3].astype(np.float32)
    g3 = gt[:, :3].astype(np.float32)

    avail = np.ones(N, dtype=bool)
    mask = np.zeros(M, dtype=bool)
    sel = np.zeros(M, dtype=np.int64)
    n_fallback = 0

    def exact_row_step(i):
        diff_i = p3[i][None, :] - g3
        d2_i = np.sum(diff_i * diff_i, axis=-1, dtype=np.float32)
        drow = np.sqrt(d2_i, dtype=np.float32)
        dm = np.where(avail, drow, np.inf)
        j = int(np.argmin(dm))
        return j, bool(dm[j] < MATCH_THRESH)

    d_l = srt_d.tolist()
    g_l = srt_g.tolist()
    f_l = floor2.tolist()
    b_l = bad.tolist()
    INF = float("inf")

    for i in range(M):
        j = -1
        ok = False
        need_fb = b_l[i]
        if not need_fb:
            row_d, row_g = d_l[i], g_l[i]
            dk, gk = INF, -1
            for k in range(len(row_g)):
                g = row_g[k]
                if g < 0:
                    break
                if avail[g]:
                    dk, gk = row_d[k], g
                    break
            if (
                gk >= 0
                and dk < MATCH_THRESH
                and dk < DILATE
                and dk * dk < f_l[i]
            ):
                j, ok = gk, True
            else:
                need_fb = True
        if need_fb:
            j, ok = exact_row_step(i)
            n_fallback += 1
        sel[i] = j
        mask[i] = ok
        if ok:
            avail[j] = False

    return mask, sel, n_fallback


# ----------------------------------------------------------------------------
# Host-side loss (reference formulas, f64)
# ----------------------------------------------------------------------------
def _host_loss(pred, gt, mask, sel):
    pb = pred.astype(np.float64)
    mg = gt[sel].astype(np.float64)
    m = mask.astype(np.float64)
    k = max(m.sum(), 1.0)

    def sl1(x):
        a = np.abs(x)
        return np.where(a < 1.0, 0.5 * a * a, a - 0.5)

    lc = (m[:, None] * sl1(pb[:, :3] - mg[:, :3])).sum() / (3 * k)
    ls = (m[:, None] * sl1(pb[:, 3:6] - mg[:, 3:6])).sum() / (3 * k)
    d = pb[:, 6] - mg[:, 6]
    d = np.arctan2(np.sin(d), np.cos(d))
    lo = (m * sl1(d)).sum() / k
    x1, y1, l1, w1 = pb[:, 0], pb[:, 1], pb[:, 3], pb[:, 4]
    x2, y2, l2, w2 = mg[:, 0], mg[:, 1], mg[:, 3], mg[:, 4]
    iw = np.clip(
        np.minimum(x1 + l1 / 2, x2 + l2 / 2) - np.maximum(x1 - l1 / 2, x2 - l2 / 2),
        0, None,
    )
    ih = np.clip(
        np.minimum(y1 + w1 / 2, y2 + w2 / 2) - np.maximum(y1 - w1 / 2, y2 - w2 / 2),
        0, None,
    )
    inter = iw * ih
    union = l1 * w1 + l2 * w2 - inter
    iou = inter / (union + 1e-6)
    li = (m * (1.0 - iou)).sum() / k
    return W_CENTER * lc + W_SIZE * (ls + lo) + W_IOU * li


# ----------------------------------------------------------------------------
# Main entry point
# ----------------------------------------------------------------------------
def kernel(pred_boxes: np.ndarray, gt_boxes: np.ndarray) -> np.ndarray:
    pred = np.ascontiguousarray(np.asarray(pred_boxes, dtype=np.float32))
    gt = np.ascontiguousarray(np.asarray(gt_boxes, dtype=np.float32))
    assert pred.shape == (M, 7) and gt.shape == (N, 7)
    core_ids = list(range(N_CORES))

    p3 = pred[:, :3].astype(np.float64)
    g3 = gt[:, :3].astype(np.float64)

    # ---- spatial blocks + per-block scanned-gt selection ----
    blocks = _lookahead_cut(p3, g3)
    insides, centers = [], []
    for blk in blocks:
        pts = p3[blk]
        lo = pts.min(axis=0) - DILATE
        hi = pts.max(axis=0) + DILATE
        insides.append(np.nonzero(((g3 >= lo) & (g3 <= hi)).all(axis=1))[0])
        centers.append(0.5 * (pts.min(axis=0) + pts.max(axis=0)))
    counts = np.array([len(x) for x in insides])
    ranked = np.argsort(-counts, kind="stable")
    # slot budgets ascending: the pipeline-fill cost is set by slot 0, so the
    # smallest blocks go first
    assign = ranked.reshape(N_SLOTS, N_CORES)[::-1]  # [slot, core] -> block id

    # per-slot budgets from the data (pad to 32; each slot PAIR shares one
    # PSUM bank, so pair sums are capped at MAX_TIER f32 columns)
    tiers = []
    for s in range(N_SLOTS):
        mx = int(counts[assign[s]].max())
        tiers.append(min(MAX_TIER - 32, max(32, -(-mx // 32) * 32)))
    for s0 in range(0, N_SLOTS, 2):
        if tiers[s0] + tiers[s0 + 1] > MAX_TIER:
            tiers[s0] = max(32, MAX_TIER - tiers[s0 + 1])
    overflow = np.zeros((N_CORES, N_SLOTS), dtype=bool)
    for s in range(N_SLOTS):
        for c in core_ids:
            if counts[assign[s, c]] > tiers[s]:
                overflow[c, s] = True
    tiers = tuple(tiers)
    off = np.concatenate([[0], np.cumsum(tiers)]).astype(int)
    C = int(off[-1])

    # ---- build per-core limb tensors ----
    idx_map = np.zeros((N_CORES, N_SLOTS, max(tiers)), dtype=np.int64)
    sent_mask = np.ones((N_CORES, N_SLOTS, max(tiers)), dtype=bool)
    in_maps = []
    import ml_dtypes

    bf = ml_dtypes.bfloat16
    for c in core_ids:
        arr = np.zeros((K_ROWS, M_PER_CORE + C), dtype=bf)
        for s in range(N_SLOTS):
            bi = assign[s, c]
            c0 = centers[bi]
            B = tiers[s]
            inside = insides[bi][:B]
            n = len(inside)
            idx_map[c, s, :n] = inside
            sent_mask[c, s, :n] = False

            # pred side: 128 preds of the block
            pc = 2.0 * (p3[blocks[bi]] - c0)          # [128, 3]
            ph, pm = _split2_bf16(pc)
            pn = -np.sum((0.5 * pc) ** 2, axis=1)     # -|p'|^2  [128]
            pnh, pnm = _split2_bf16(pn)
            colp = slice(s * 128, (s + 1) * 128)
            for cc in range(3):
                arr[cc * 4 + 0, colp] = ph[:, cc]
                arr[cc * 4 + 1, colp] = ph[:, cc]
                arr[cc * 4 + 2, colp] = pm[:, cc]
                arr[cc * 4 + 3, colp] = pm[:, cc]
            arr[12, colp] = 1.0
            arr[13, colp] = 1.0
            arr[14, colp] = pnh
            arr[15, colp] = pnm

            # gt side: scanned gts then sentinels
            gc = np.full((B, 3), SENT_OFF, dtype=np.float64)
            gc[:n] = g3[inside] - c0
            gh, gm = _split2_bf16(gc)
            gn = -np.sum(gc * gc, axis=1)             # -|g'|^2  [B]
            gnh, gnm = _split2_bf16(gn)
            colg = slice(M_PER_CORE + off[s], M_PER_CORE + off[s] + B)
            for cc in range(3):
                arr[cc * 4 + 0, colg] = gh[:, cc]
                arr[cc * 4 + 1, colg] = gm[:, cc]
                arr[cc * 4 + 2, colg] = gh[:, cc]
                arr[cc * 4 + 3, colg] = gm[:, cc]
            arr[12, colg] = gnh
            arr[13, colg] = gnm
            arr[14, colg] = 1.0
            arr[15, colg] = 1.0
        in_maps.append({"limbs": np.ascontiguousarray(arr)})

    perm = np.concatenate(
        [blocks[assign[s, c]] for c in core_ids for s in range(N_SLOTS)]
    )

    # ---- device launch ----
    nc = _get_program(tiers)
    res = run_bass_kernel_spmd(nc, in_maps, core_ids, trace=TRACE)
    LAST_EXEC_NS["phase1"] = res.exec_time_ns

    import ml_dtypes as _mld

    NV = N_SLOTS * K_CAND
    vals_p = np.concatenate(
        [
            np.ascontiguousarray(res.results[c]["out"][:, :NV])
            .view(_mld.bfloat16)
            .reshape(128, N_SLOTS, K_CAND)
            .transpose(1, 0, 2)
            .reshape(M_PER_CORE, K_CAND)
            for c in core_ids
        ],
        axis=0,
    )
    idxs_p = np.concatenate(
        [
            res.results[c]["out"][:, NV:]
            .reshape(128, N_SLOTS, K_CAND)
            .transpose(1, 0, 2)
            .reshape(M_PER_CORE, K_CAND)
            for c in core_ids
        ],
        axis=0,
    )

    # ---- decode: expand each winner into its 4 alias columns ----
    core_of_row = np.repeat(np.arange(N_CORES), M_PER_CORE)
    slot_of_row = np.tile(np.repeat(np.arange(N_SLOTS), 128), N_CORES)
    tiers_arr = np.array(tiers)
    q_of_row = tiers_arr[slot_of_row] // 4                    # [M]
    loc_raw = idxs_p.astype(np.int64)                         # [M, 8] in [0,Q)
    loc = np.minimum(loc_raw, q_of_row[:, None] - 1)
    alias = loc[:, :, None] + np.arange(N_ALIAS)[None, None, :] * q_of_row[
        :, None, None
    ]                                                          # [M, 8, 4]
    gids = idx_map[core_of_row[:, None, None], slot_of_row[:, None, None], alias]
    sent = sent_mask[core_of_row[:, None, None], slot_of_row[:, None, None], alias]
    sent |= (loc_raw != loc)[:, :, None]

    p3f = pred[:, :3].astype(np.float32)
    g3f = gt[:, :3].astype(np.float32)
    diffc = p3f[perm][:, None, None, :] - g3f[gids]
    d2c = np.sum(diffc * diffc, axis=-1, dtype=np.float32)
    dc = np.sqrt(d2c, dtype=np.float32)
    dc[sent] = np.inf
    gids_s = np.where(sent, -1, gids)

    # empirical score-error bound: approx val vs exact best-alias score.
    # bf16 staging makes the error value-relative: err <= e_abs + REL*|val|.
    vap = vals_p.astype(np.float64)                            # approx max score
    d2min = np.min(np.where(sent, np.inf, d2c.astype(np.float64)), axis=2)
    real = np.isfinite(d2min) & (vap > -1.0e8)
    err = np.abs(np.where(real, -vap - d2min, 0.0))
    REL = 2.0 ** -8
    e_abs = float(np.maximum(err - REL * np.abs(vap), 0.0).max())
    DIAG["eps_abs"] = e_abs

    # flatten to [M, 32] sorted by (exact distance, gt id)
    dflat = dc.reshape(M, K_CAND * N_ALIAS)
    gflat = gids_s.reshape(M, K_CAND * N_ALIAS)
    order = np.lexsort((gflat, dflat), axis=-1)
    srt_d = np.take_along_axis(dflat, order, axis=-1)
    srt_g = np.take_along_axis(gflat, order, axis=-1)
    srt_g[~np.isfinite(srt_d)] = -1
    # move sentinels (-1 gid) to the end marker-wise: walk breaks at g<0, so
    # ensure no real candidate sorts after a sentinel (inf distance => last).
    v7 = vap[:, K_CAND - 1]
    eps_row = 1.3 * (e_abs + REL * np.abs(v7)) + 1e-3
    floor2 = np.maximum(-v7 - eps_row, 0.0)
    bad = overflow[core_of_row, slot_of_row]

    # back to original pred order
    inv = np.empty(M, dtype=np.int64)
    inv[perm] = np.arange(M)
    srt_d = srt_d[inv]
    srt_g = srt_g[inv]
    floor2 = floor2[inv]
    bad = bad[inv]

    t_walk = _time.time()
    mask, sel, n_fb = _host_greedy(pred, gt, srt_d, srt_g, floor2, bad)
    DIAG["n_fallback"] = n_fb
    DIAG["n_overflow_blocks"] = int(overflow.sum())
    DIAG["t_walk"] = _time.time() - t_walk
    DIAG["tiers"] = tiers

    loss = _host_loss(pred, gt, mask, sel)
    return np.float32(loss)
